# revision 14
# baseline (speedup 1.0000x reference)
"""Trainium2 Bass kernel for the BKT (multi-HMM knowledge tracing) forward model.

Reformulated recursion (validated in proto.py):
  state  γ(t) = α(t) − a3(t−1)            [128 students × (s,k) = 200]
  chain  x4(t) = u4(t) + κ_{t−1}·a3(t−1)  [stt w4, DVE]
         ps2(t)[s'] = Σ_s exp(x4[s,s'])   [2× ACT Exp w2 with accum_out]
         a3(t) = ln(ps2)                  [ACT Ln w2]
  off    pd(t) = a3(t−1) − a3(t)          [Pool tt w2]
         γ(t+1) = cinv_t ⊙ (γ(t)+pd(t))   [2× DVE stt w100]
         r'(t+2) = <Q_{t+1}, γ(t+1)> + σ_{t+1}·a3(t)
             [2× DVE affine_mul_reduce: accum = Σ(γ+a3)·Q = <Q,γ> + a3·ΣQ
              and ΣQ_{t+1} = σ_{t+1}, so the seed rides the bias slot]
         u4(t+2) = w4(t+2) + r'(t+2)      [Pool tt w4]
  where Q_t = c_{t+1}⊙(1−c_t), σ_t = ΣQ_t, κ_t = 1−σ_t, all host streams.
  (tensor_tensor_reduce and ACT accum_out crash this runtime — avoided.)
Outputs deferred to one batched tail:
  D(j) = (r'_1−r'_0)(j) + κ_{j−1}·(a3_1−a3_0)(j−1),
  out[t,o] = L0[t,o] + ln(1+e^{dL[t,o]+D}) − ln(1+e^{D}).
Step 0/1 warm-started on host (uploads γ(1), a3(0), u4(1..2), r'(2), D(0..1)).
No gathers: Q/cinv streamed dense from HBM, chunked + double buffered.
"""

import os
from contextlib import ExitStack

import numpy as np

N_PROBLEMS = 10000
N_KCS = 100
BATCH = 1024
T_FULL = 500
N_CORES = 8
BL = BATCH // N_CORES  # 128 students per core

_CH = 32       # time steps per Q/cinv stream chunk
_PREF = 2      # chunks prefetched ahead
_SMALL_ON_POOL = True


def _log_softmax(x, axis):
    x = x.astype(np.float64)
    m = x.max(axis=axis, keepdims=True)
    e = np.exp(x - m)
    return (x - m) - np.log(e.sum(axis=axis, keepdims=True))


def _setup_act_tables():
    """Force the 'natural_log_exp_and_others' ACT table set to be chosen for
    both Exp and Ln so no ACT_TABLE_LOAD appears mid-loop."""
    import glob
    import json
    import tempfile

    if os.environ.get("_BKT_ACT_TABLES"):
        return
    from neuronxcc.driver.Job import Job  # pyright: ignore[reportMissingImports]
    from neuronxcc.driver.jobs.support.FindActInfo import (  # pyright: ignore[reportMissingImports]
        findActInfoFile,
    )

    src = findActInfoFile(Job.getPackageDir(), "gen3")
    d = json.load(open(src))
    d["act_func_sets"] = sorted(
        d["act_func_sets"],
        key=lambda s: s["name"] != "natural_log_exp_and_others")
    tmp = tempfile.mkdtemp(prefix="bkt_act_")
    with open(tmp + "/act_info.json", "w") as f:
        json.dump(d, f)
    for p in glob.glob(os.path.dirname(src) + "/*"):
        b = os.path.basename(p)
        if b != "act_info.json":
            os.symlink(p, tmp + "/" + b)
    os.environ["BASS_ACT_ROOT_JSON_PATH"] = tmp + "/act_info.json"
    os.environ["_BKT_ACT_TABLES"] = "1"

    import concourse.bacc as bacc_mod
    import concourse.mybir as mybir

    def tables(arch):
        return {
            e["name"]: {mybir.ActivationFunctionType.from_pwp(v)
                        for v in e["act"].keys()}
            for e in d["act_func_sets"]
        }

    bacc_mod.get_activation_tables = tables


def _emit_program(T):
    import concourse.mybir as mybir
    import concourse.tile as tile
    from concourse import bacc, library_config

    _setup_act_tables()

    f32 = mybir.dt.float32
    Alu = mybir.AluOpType
    Act = mybir.ActivationFunctionType
    K = N_KCS

    nc = bacc.Bacc("TRN2", target_bir_lowering=False, debug=False)

    # DRAM inputs
    qs = nc.dram_tensor("qs", [BL, T * K], f32, kind="ExternalInput")
    cinvs = nc.dram_tensor("cinvs", [BL, T * K], f32, kind="ExternalInput")
    w4s = nc.dram_tensor("w4s", [BL, 4 * T], f32, kind="ExternalInput")
    kaps = nc.dram_tensor("kaps", [BL, T], f32, kind="ExternalInput")
    l0s_d = nc.dram_tensor("l0s", [BL, 2 * T], f32, kind="ExternalInput")
    dls_d = nc.dram_tensor("dls", [BL, 2 * T], f32, kind="ExternalInput")
    gammaw = nc.dram_tensor("gammaw", [BL, 2 * K], f32, kind="ExternalInput")
    a30w = nc.dram_tensor("a30w", [BL, 2], f32, kind="ExternalInput")
    u4w = nc.dram_tensor("u4w", [BL, 8], f32, kind="ExternalInput")
    rp2w = nc.dram_tensor("rp2w", [BL, 2], f32, kind="ExternalInput")
    d01w = nc.dram_tensor("d01w", [BL, 2], f32, kind="ExternalInput")
    out_d = nc.dram_tensor("out", [BL, 2 * T], f32, kind="ExternalOutput")

    n_chunks = (T + _CH - 1) // _CH

    with ExitStack() as ctx:
        tc = ctx.enter_context(tile.TileContext(nc))
        if _SMALL_ON_POOL:
            nc.gpsimd.load_library(library_config.standard)

        fixed = ctx.enter_context(tc.tile_pool(name="fixed", bufs=1))
        slabp = ctx.enter_context(tc.tile_pool(name="slabs", bufs=_PREF + 1))
        gpool = ctx.enter_context(tc.tile_pool(name="gamma", bufs=2))
        x4p = ctx.enter_context(tc.tile_pool(name="x4", bufs=3))
        sm = ctx.enter_context(tc.tile_pool(name="sm", bufs=3))

        # --- fixed tiles + warm uploads (issue order = DMA drain order) ---
        gamma = gpool.tile([BL, 2 * K], f32, tag="gamma")
        nc.sync.dma_start(gamma[:], gammaw.ap())
        a3buf = fixed.tile([BL, 2 * T], f32)
        nc.sync.dma_start(a3buf[:, 0:2], a30w.ap())
        u4buf = fixed.tile([BL, 4 * T], f32)
        nc.sync.dma_start(u4buf[:, 4:12], u4w.ap())
        r2buf = fixed.tile([BL, 2 * T], f32)
        nc.sync.dma_start(r2buf[:, 4:6], rp2w.ap())
        kap = fixed.tile([BL, T], f32)
        nc.sync.dma_start(kap[:], kaps.ap())

        qslab = [None] * n_chunks
        cislab = [None] * n_chunks

        def issue_chunk(n):
            t0 = n * _CH
            w = min(_CH, T - t0) * K
            qt = slabp.tile([BL, _CH, K], f32, tag="qsl")
            nc.sync.dma_start(qt[:].rearrange("p a b -> p (a b)")[:, 0:w],
                              qs.ap()[:, t0 * K:t0 * K + w])
            ct = slabp.tile([BL, _CH, K], f32, tag="cisl")
            nc.sync.dma_start(ct[:].rearrange("p a b -> p (a b)")[:, 0:w],
                              cinvs.ap()[:, t0 * K:t0 * K + w])
            qslab[n], cislab[n] = qt, ct

        for n in range(min(_PREF, n_chunks)):
            issue_chunk(n)

        w4b = fixed.tile([BL, 4 * T], f32)
        nc.sync.dma_start(w4b[:], w4s.ap())
        Dbuf = fixed.tile([BL, T], f32)
        nc.sync.dma_start(Dbuf[:, 0:2], d01w.ap())
        l0b = fixed.tile([BL, 2 * T], f32)
        nc.sync.dma_start(l0b[:], l0s_d.ap())
        dlb = fixed.tile([BL, 2 * T], f32)
        nc.sync.dma_start(dlb[:], dls_d.ap())

        junk = fixed.tile([BL, 2 * K], f32)

        smalls = nc.gpsimd if _SMALL_ON_POOL else nc.vector

        # --- main loop: chain t = 1..T-2, off-chain t = 1..T-3 ---
        for t in range(1, T - 1):
            n = t // _CH
            m = n + _PREF - 1
            if t % _CH == 0 and m < n_chunks and qslab[m] is None:
                issue_chunk(m)

            # CHAIN: x4(t) = κ_{t-1}·a3(t-1) + u4(t)
            x4t = x4p.tile([BL, 4], f32, tag="x4")
            nc.vector.scalar_tensor_tensor(
                out=x4t[:].rearrange("p (s sp) -> p s sp", s=2),
                in0=a3buf[:, 2 * (t - 1):2 * t]
                    .rearrange("p (s o) -> p s o", s=2).broadcast_to([BL, 2, 2]),
                scalar=kap[:, t:t + 1],
                in1=u4buf[:, 4 * t:4 * t + 4].rearrange("p (s sp) -> p s sp", s=2),
                op0=Alu.mult, op1=Alu.add,
            )
            e4t = sm.tile([BL, 4], f32, tag="e4")
            nc.scalar.activation(e4t[:], x4t[:], Act.Exp)
            ps2t = sm.tile([BL, 2], f32, tag="ps2")
            nc.vector.tensor_tensor(
                out=ps2t[:], in0=e4t[:, 0:2], in1=e4t[:, 2:4], op=Alu.add)
            nc.scalar.activation(a3buf[:, 2 * t:2 * t + 2], ps2t[:], Act.Ln)

            if t > T - 3:
                continue
            # OFF-CHAIN
            pd2t = sm.tile([BL, 2], f32, tag="pd2")
            smalls.tensor_tensor(
                out=pd2t[:], in0=a3buf[:, 2 * (t - 1):2 * t],
                in1=a3buf[:, 2 * t:2 * t + 2], op=Alu.subtract)
            gnew = gpool.tile([BL, 2 * K], f32, tag="gamma")
            ci = cislab[n][:, t % _CH, :]
            qv = qslab[n][:, t % _CH, :]
            for s in range(2):
                nc.vector.scalar_tensor_tensor(
                    out=gnew[:, s * K:(s + 1) * K],
                    in0=gamma[:, s * K:(s + 1) * K],
                    scalar=pd2t[:, s:s + 1],
                    in1=ci, op0=Alu.add, op1=Alu.mult,
                )
            for s in range(2):
                # accum = Σ(γ_s + a3_s)·Q = <Q,γ_s> + σ·a3_s  (ΣQ = σ)
                nc.vector.affine_mul_reduce(
                    out=junk[:, s * K:(s + 1) * K],
                    accum_out=r2buf[:, 2 * (t + 2) + s:2 * (t + 2) + s + 1],
                    in0=gnew[:, s * K:(s + 1) * K], in1=qv,
                    scale=1.0, bias=a3buf[:, 2 * t + s:2 * t + s + 1],
                )
            smalls.tensor_tensor(
                out=u4buf[:, 4 * (t + 2):4 * (t + 2) + 4]
                    .rearrange("p (s sp) -> p s sp", s=2),
                in0=w4b[:, 4 * (t + 2):4 * (t + 2) + 4]
                    .rearrange("p (s sp) -> p s sp", s=2),
                in1=r2buf[:, 2 * (t + 2):2 * (t + 2) + 2]
                    .rearrange("p (s o) -> p s o", s=2).broadcast_to([BL, 2, 2]),
                op=Alu.add,
            )
            gamma = gnew

        # --- deferred output tail ---
        da = fixed.tile([BL, T], f32)
        a3v = a3buf[:].rearrange("p (t s) -> p t s", s=2)
        nc.vector.tensor_tensor(
            out=da[:, 1:T - 1].rearrange("p (t o) -> p t o", o=1),
            in0=a3v[:, 1:T - 1, 1:2], in1=a3v[:, 1:T - 1, 0:1],
            op=Alu.subtract)
        dr = fixed.tile([BL, T], f32)
        r2v = r2buf[:].rearrange("p (t s) -> p t s", s=2)
        nc.vector.tensor_tensor(
            out=dr[:, 2:T].rearrange("p (t o) -> p t o", o=1),
            in0=r2v[:, 2:T, 1:2], in1=r2v[:, 2:T, 0:1], op=Alu.subtract)
        m1 = fixed.tile([BL, T], f32)
        nc.vector.tensor_tensor(
            out=m1[:, 2:T], in0=kap[:, 2:T], in1=da[:, 1:T - 1], op=Alu.mult)
        nc.vector.tensor_tensor(
            out=Dbuf[:, 2:T], in0=dr[:, 2:T], in1=m1[:, 2:T], op=Alu.add)
        yD = fixed.tile([BL, 2 * T], f32)
        nc.vector.tensor_tensor(
            out=yD[:].rearrange("p (t o) -> p t o", o=2),
            in0=dlb[:].rearrange("p (t o) -> p t o", o=2),
            in1=Dbuf[:].rearrange("p (t o) -> p t o", o=1)
                .broadcast_to([BL, T, 2]),
            op=Alu.add)
        e2 = fixed.tile([BL, 2 * T], f32)
        nc.scalar.activation(e2[:], yD[:], Act.Exp)
        l2 = fixed.tile([BL, 2 * T], f32)
        nc.scalar.activation(l2[:], e2[:], Act.Ln, bias=1.0)
        ed = fixed.tile([BL, T], f32)
        nc.scalar.activation(ed[:], Dbuf[:], Act.Exp)
        ld = fixed.tile([BL, T], f32)
        nc.scalar.activation(ld[:], ed[:], Act.Ln, bias=1.0)
        outb = fixed.tile([BL, 2 * T], f32)
        nc.vector.tensor_tensor(
            out=outb[:].rearrange("p (t o) -> p t o", o=2),
            in0=l2[:].rearrange("p (t o) -> p t o", o=2),
            in1=ld[:].rearrange("p (t o) -> p t o", o=1)
                .broadcast_to([BL, T, 2]),
            op=Alu.subtract)
        nc.vector.tensor_tensor(out=outb[:], in0=outb[:], in1=l0b[:], op=Alu.add)
        nc.sync.dma_start(out_d.ap(), outb[:])

    nc.compile()
    return nc


def _prep_inputs(corr, kc, problem, A, trans_logits, obs_logits_problem,
                 init_logits, T):
    corr = np.asarray(corr).astype(np.int64)[:, :T]
    kc = np.asarray(kc).astype(np.int64)[:, :T]
    problem = np.asarray(problem).astype(np.int64)[:, :T]
    A = np.asarray(A).astype(np.float64)
    K = N_KCS

    log_t = _log_softmax(np.asarray(trans_logits), axis=1)
    G = A @ log_t.reshape(K, 4)                       # [P,4], j = 2 s' + s
    L = _log_softmax(np.asarray(obs_logits_problem), axis=2)
    la0 = _log_softmax(np.asarray(init_logits), axis=1)

    in_maps = []
    for i in range(N_CORES):
        sl = slice(i * BL, (i + 1) * BL)
        kc_l, pp_l, cr_l = kc[sl], problem[sl], corr[sl]
        c = A[kc_l]                                   # [BL,T,K]
        cinv = 1.0 - c
        Q = c[:, 1:] * cinv[:, :-1]                   # Q[:,j] = c_{j+1}*cinv_j
        sigma = Q.sum(-1)
        kappa = 1.0 - sigma
        OLL = np.take_along_axis(
            L[pp_l], cr_l[:, :, None, None], axis=3)[:, :, :, 0]  # [BL,T,2]
        Gk = G[kc_l]
        w4 = np.stack([Gk[..., 0] + OLL[..., 0], Gk[..., 2] + OLL[..., 0],
                       Gk[..., 1] + OLL[..., 1], Gk[..., 3] + OLL[..., 1]],
                      axis=2)                         # [BL,T,4] (s,s')
        Lp = L[pp_l]
        l0 = np.ascontiguousarray(Lp[:, :, 0, :]).reshape(BL, 2 * T)
        dl = np.ascontiguousarray(Lp[:, :, 1, :] - Lp[:, :, 0, :]).reshape(BL, 2 * T)

        # warm start
        alpha0 = np.broadcast_to(la0.T.reshape(1, 2, K), (BL, 2, K))
        a2_0 = np.einsum('bk,bsk->bs', c[:, 0], alpha0)
        x40 = w4[:, 0].reshape(BL, 2, 2) + a2_0[:, :, None]
        a30 = np.log(np.exp(x40).sum(axis=1))
        alpha1 = cinv[:, 0][:, None, :] * alpha0 + c[:, 0][:, None, :] * a30[:, :, None]
        gamma1 = alpha1 - a30[:, :, None]
        a2_1 = np.einsum('bk,bsk->bs', c[:, 1], gamma1) + a30
        u4_1 = w4[:, 1].reshape(BL, 2, 2) + a2_1[:, :, None] \
            - kappa[:, 0][:, None, None] * a30[:, :, None]
        rp2 = np.einsum('bk,bsk->bs', Q[:, 1], gamma1) + sigma[:, 1][:, None] * a30
        u4_2 = w4[:, 2].reshape(BL, 2, 2) + rp2[:, :, None]

        # iteration-shifted streams
        kap_it = np.zeros((BL, T)); kap_it[:, 1:] = kappa[:, 0:T - 1]
        q_it = np.zeros((BL, T, K)); q_it[:, 1:T - 2] = Q[:, 2:T - 1]
        cinv_it = np.zeros((BL, T, K)); cinv_it[:, 1:T - 2] = cinv[:, 1:T - 2]

        f = np.float32
        im = {
            "qs": q_it.reshape(BL, T * K).astype(f),
            "cinvs": cinv_it.reshape(BL, T * K).astype(f),
            "w4s": w4.reshape(BL, 4 * T).astype(f),
            "kaps": kap_it.astype(f),
            "l0s": l0.astype(f),
            "dls": dl.astype(f),
            "gammaw": gamma1.reshape(BL, 2 * K).astype(f),
            "a30w": a30.astype(f),
            "u4w": np.concatenate([u4_1.reshape(BL, 4), u4_2.reshape(BL, 4)],
                                  axis=1).astype(f),
            "rp2w": rp2.astype(f),
            "d01w": np.stack([a2_0[:, 1] - a2_0[:, 0],
                              a2_1[:, 1] - a2_1[:, 0]], axis=1).astype(f),
        }
        in_maps.append(im)
    return in_maps


def kernel(corr, kc, problem, A, trans_logits, obs_logits_problem, init_logits,
           _T=None, _trace=False):
    T = _T or T_FULL
    nc = _emit_program(T)
    in_maps = _prep_inputs(corr, kc, problem, A, trans_logits,
                           obs_logits_problem, init_logits, T)

    from concourse.bass_utils import run_bass_kernel_spmd
    res = run_bass_kernel_spmd(nc, in_maps, core_ids=list(range(N_CORES)),
                               trace=_trace)
    outs = [r["out"].reshape(BL, T, 2) for r in res.results]
    full = np.concatenate(outs, axis=0).astype(np.float32)
    kernel.last_results = res
    return full


if __name__ == "__main__":
    pass


# revision 15
# speedup vs baseline: 1.1064x; 1.1064x over previous
"""Trainium2 Bass kernel for the BKT (multi-HMM knowledge tracing) forward model.

Strategy: data-parallel over students (1024 students / 8 cores = 128 per core,
one SBUF partition per student). The T=500 time recursion runs locally per core.

Per-core algebra per step t:
    c      = A[kc[:,t]]                          [128,100]  (gathered, rows sum to 1)
    strm   = [w5_s0 | w5_s1] per (student,t)     [128,10]   (host-precomputed stream)
             where w5_s = [M4[s], M4[2+s], L4[s], L4[2+s], 0],
             M4 = (A @ log_t)[kc] + OLL(s') and L4 = log_obs[problem]
    a2_s   = sum_k c * alpha_s     (fused scalar_tensor_tensor accum; byproduct
                                    u_s = c * alpha_s is kept)
    e_s    = exp(strm_s + a2_s)                  (activation with bias=a2_s)
    ps5    = e_0 + e_1 = [se0,se1,po0,po1,q]
    lg5    = ln(ps5): a3 = lg5[:,0:2], log_py = lg5[:,2:4] - lg5[:,4:5]
    v_s    = alpha_s - u_s  (= alpha*(1-c), off the critical chain)
    alpha_s' = c * a3_s + v_s                    (one fused stt per s)

Only the A rows (indexed by kc, 512B each) are gathered on device; the small
per-step observation data (10 f32) is a pure function of the inputs and is
streamed in dense [128, T, 10] layout (loaded once into SBUF). Gathers are
spread across all 4 SWDGE queues with deep prefetch so descriptor generation
and DMA drain hide under the vector-engine recursion.

The predicted-output normalizer q = sum_o po_o collapses to e^{a2_0}+e^{a2_1}
because log_obs is normalized over o; sum_k c = 1 because A rows are a softmax.
"""

import os
from contextlib import ExitStack

import numpy as np

N_PROBLEMS = 10000
N_KCS = 100
BATCH = 1024
T_FULL = 500
N_CORES = 8
BL = BATCH // N_CORES  # 128 students per core

_CHUNK = 8  # time steps per gather slab (8*128 = 1024 = SWDGE ring capacity)
_PREFETCH = 5  # chunks of gather issued ahead of consumption
_NQ = 4  # SWDGE queues used round-robin
_BF16 = False  # keep alpha/c/u/v state in bf16 (DVE 2x mode); exp/ln stay fp32


def _log_softmax(x, axis):
    x = x.astype(np.float32)
    m = x.max(axis=axis, keepdims=True)
    e = np.exp(x - m)
    return (x - m) - np.log(e.sum(axis=axis, keepdims=True))


def _wrap_idx(flat):
    """dma_gather index layout: flat index i lives at partition i%16, col i//16,
    replicated across the 8 gpsimd cores (16-partition groups)."""
    assert flat.size % 16 == 0
    w = flat.astype(np.int16).reshape(-1, 16).T  # [16, N/16]
    return np.tile(w, (8, 1))  # [128, N/16]


def _host_tables(A, trans_logits, obs_logits_problem, init_logits):
    import ml_dtypes
    P = A.shape[0]
    K = trans_logits.shape[0]
    sdt = ml_dtypes.bfloat16 if _BF16 else np.float32
    log_t = _log_softmax(trans_logits, axis=1)  # [K,2,2] normalized over middle axis
    G = A.astype(np.float32) @ log_t.reshape(K, 4)  # [P,4] laid out (s,s')
    L = _log_softmax(obs_logits_problem, axis=2)  # [P,2,2] normalized over outputs

    taba = np.zeros((P, 128), sdt)
    taba[:, 0:100] = A.astype(sdt)

    la0 = _log_softmax(init_logits, axis=1)  # [K,2]
    alpha0 = np.empty((BL, 2 * K), sdt)
    alpha0[:, 0:K] = la0[:, 0].astype(sdt)  # s=0 block
    alpha0[:, K:] = la0[:, 1].astype(sdt)  # s=1 block
    return taba, G, L, alpha0


def _setup_act_tables():
    """Both Exp and Ln live in the 'natural_log_exp_and_others' ACT table
    set, but the default set ordering makes bacc pick a different set for
    each, inserting a ~2.7us ACT_TABLE_LOAD per activation (2 per time
    step!). Reorder the set list so that set comes first for both bacc's
    chooser and walrus (via BASS_ACT_ROOT_JSON_PATH), collapsing the loads
    to one for the whole kernel."""
    import glob
    import json
    import tempfile

    if os.environ.get("_BKT_ACT_TABLES"):
        return
    from neuronxcc.driver.Job import Job  # pyright: ignore[reportMissingImports]
    from neuronxcc.driver.jobs.support.FindActInfo import (  # pyright: ignore[reportMissingImports]
        findActInfoFile,
    )

    src = findActInfoFile(Job.getPackageDir(), "gen3")
    d = json.load(open(src))
    d["act_func_sets"] = sorted(
        d["act_func_sets"],
        key=lambda s: s["name"] != "natural_log_exp_and_others")
    tmp = tempfile.mkdtemp(prefix="bkt_act_")
    with open(tmp + "/act_info.json", "w") as f:
        json.dump(d, f)
    for p in glob.glob(os.path.dirname(src) + "/*"):
        b = os.path.basename(p)
        if b != "act_info.json":
            os.symlink(p, tmp + "/" + b)
    os.environ["BASS_ACT_ROOT_JSON_PATH"] = tmp + "/act_info.json"
    os.environ["_BKT_ACT_TABLES"] = "1"

    import concourse.bacc as bacc_mod
    import concourse.mybir as mybir

    def tables(arch):
        return {
            e["name"]: {mybir.ActivationFunctionType.from_pwp(v)
                        for v in e["act"].keys()}
            for e in d["act_func_sets"]
        }

    bacc_mod.get_activation_tables = tables


def _emit_program(T, Tc):
    import concourse.mybir as mybir
    import concourse.tile as tile
    from concourse import bacc

    _setup_act_tables()

    f32 = mybir.dt.float32
    sdt = mybir.dt.bfloat16 if _BF16 else f32
    i16 = mybir.dt.int16
    Alu = mybir.AluOpType
    Act = mybir.ActivationFunctionType
    K = N_KCS

    nc = bacc.Bacc("TRN2", target_bir_lowering=False, debug=False,
                   num_swdge_queues=_NQ)

    Th = min(2 * Tc, T)  # head steps staged in separate dense tensors
    n_dense = min(_PREFETCH, (T + Tc - 1) // Tc)  # host-pregathered chunks
    taba = nc.dram_tensor("taba", [N_PROBLEMS, 128], sdt, kind="ExternalInput")
    strm = nc.dram_tensor("strm", [BL, T * 10], f32, kind="ExternalInput")
    kcw = nc.dram_tensor("kcw", [128, T * 8], i16, kind="ExternalInput")
    strmh = nc.dram_tensor("strmh", [BL, Th * 10], f32, kind="ExternalInput")
    alpha0 = nc.dram_tensor("alpha0", [BL, 2 * K], sdt, kind="ExternalInput")
    # 2-step head of chunk 0 as its own tensor: lands first so step 0's a2
    # doesn't wait for the whole first slab
    Td = min(2, T)
    def _sd_w(n):
        tcn = min(Tc, T - n * Tc)
        return tcn - Td if (n == 0 and tcn > Td) else tcn

    slabd = [nc.dram_tensor(f"slabd{n}", [128, _sd_w(n) * 128], sdt,
                            kind="ExternalInput")
             for n in range(n_dense)]
    slabd0h = nc.dram_tensor("slabd0h", [128, Td * 128], sdt,
                             kind="ExternalInput")
    out = nc.dram_tensor("out", [BL, T * 2], f32, kind="ExternalOutput")

    # SWDGE descriptor ring fits 1024 descriptors; each gathered row is one
    # descriptor, so cap each dma_gather call at 1024 indices (8 steps).
    assert Tc * 128 <= 1024
    chunks = []  # (t0, tcn)
    t0 = 0
    while t0 < T:
        chunks.append((t0, min(Tc, T - t0)))
        t0 += Tc
    n_chunks = len(chunks)

    from concourse import library_config

    with ExitStack() as ctx:
        tc = ctx.enter_context(tile.TileContext(nc))
        nc.gpsimd.load_library(library_config.mlp)
        idx_pool = ctx.enter_context(tc.tile_pool(name="idx", bufs=1))
        slab_pool = ctx.enter_context(
            tc.tile_pool(name="slabs", bufs=_PREFETCH + 1))
        state_pool = ctx.enter_context(tc.tile_pool(name="state", bufs=2))
        small_pool = ctx.enter_context(tc.tile_pool(name="small", bufs=4))
        u_pool = ctx.enter_context(tc.tile_pool(name="u", bufs=2))
        out_pool = ctx.enter_context(tc.tile_pool(name="outb", bufs=1))

        # DMA issue order = DMA queue drain order: step 0's a2 needs only
        # alpha + the 2-step slab head (emitted first inside issue_gather(0)),
        # then strmh for the first Exps; the bulk strm/kcw loads (needed from
        # t=2*Tc / chunk _PREFETCH onward) drain last.
        alpha = state_pool.tile([128, 2 * K], sdt, tag="alpha")
        nc.sync.dma_start(alpha[:], alpha0.ap())
        strmh_t = idx_pool.tile([128, Th * 10], f32, tag="strmh")
        nc.sync.dma_start(strmh_t[:], strmh.ap())
        kcw_t = idx_pool.tile([128, T * 8], i16, tag="kcw")
        strm_t = idx_pool.tile([128, T * 10], f32, tag="strm")

        outbuf = out_pool.tile([128, T * 2], f32)
        # per-step ln() results land here: [se0, se1, po0, po1, q] per t
        lgbuf = out_pool.tile([128, T * 5], f32)

        slabsA = [None] * n_chunks
        ni_regs = {}  # distinct chunk sizes get one register each
        for tcn in sorted({c[1] for c in chunks}):
            r = nc.gpsimd.alloc_register(f"ni{tcn}")
            nc.gpsimd.reg_mov(r, tcn * 128)
            ni_regs[tcn] = r

        def issue_gather(n):
            t0, tcn = chunks[n]
            sa = slab_pool.tile([128, Tc, 128], sdt, tag="slabA")
            if n < len(slabd):
                # first chunks come host-pregathered as dense DMAs so the
                # recursion starts without waiting for the gpsimd library
                # load + first dma_gather (~27us of startup otherwise)
                if n == 0 and Td < tcn:
                    # chunk 0 split: 2-step head first, rest behind it
                    nc.sync.dma_start(sa[:, 0:Td, :], slabd0h.ap())
                    nc.sync.dma_start(sa[:, Td:tcn, :], slabd[n].ap())
                else:
                    nc.sync.dma_start(sa[:, 0:tcn, :], slabd[n].ap())
            else:
                nc.gpsimd.dma_gather(
                    sa[:, 0:tcn, :], taba.ap(),
                    kcw_t[:, t0 * 8:(t0 + tcn) * 8],
                    num_idxs=tcn * 128, num_idxs_reg=ni_regs[tcn],
                    elem_size=128, queue_num=n % _NQ,
                )
            slabsA[n] = sa

        for n in range(min(_PREFETCH, n_chunks)):
            issue_gather(n)

        # bulk loads issue after the startup-critical DMAs above
        nc.sync.dma_start(strm_t[:], strm.ap())
        nc.sync.dma_start(kcw_t[:], kcw.ap())

        def c_ap(t):
            return slabsA[t // Tc][:, t % Tc, 0:K]

        def emit_a2(t, alpha_t):
            """a2_s = sum_k c * alpha_s, fused via scalar_tensor_tensor accum.
            alpha is blocked [128, (s k)]; slices pick each s. Returns
            (a2, u2) where u2 = c * alpha is reused for the state update."""
            a2 = small_pool.tile([128, 2], f32, tag="a2")
            u2 = u_pool.tile([128, 2 * K], sdt, tag="u2")
            for s in range(2):
                nc.vector.scalar_tensor_tensor(
                    out=u2[:, s * K:(s + 1) * K], in0=c_ap(t), scalar=0.0,
                    in1=alpha_t[:, s * K:(s + 1) * K],
                    op0=Alu.bypass, op1=Alu.mult,
                    accum_out=a2[:, s:s + 1],
                )
            return a2, u2

        # prologue: a2 for t=0
        a2, u2 = emit_a2(0, alpha)

        for n in range(n_chunks):
            if n + _PREFETCH < n_chunks:
                issue_gather(n + _PREFETCH)
            for j in range(chunks[n][1]):
                t = chunks[n][0] + j
                # e_s = exp(strm_s + a2_s); e_0 only waits on a2_0
                st = strmh_t if t < Th else strm_t
                e10 = small_pool.tile([128, 10], f32, tag="e10")
                for s in range(2):
                    nc.scalar.activation(
                        e10[:, 5 * s:5 * s + 5],
                        st[:, 10 * t + 5 * s:10 * t + 5 * s + 5],
                        Act.Exp, bias=a2[:, s:s + 1],
                    )
                # v = alpha*(1-c), off the Ln chain (runs under the Exps)
                v = u_pool.tile([128, 2 * K], sdt, tag="v")
                nc.vector.tensor_tensor(
                    out=v[:], in0=alpha[:], in1=u2[:], op=Alu.subtract,
                )
                ps5 = small_pool.tile([128, 5], f32, tag="ps5")
                nc.vector.tensor_tensor(
                    out=ps5[:], in0=e10[:, 0:5], in1=e10[:, 5:10], op=Alu.add,
                )
                lg5 = lgbuf[:, 5 * t:5 * t + 5]
                nc.scalar.activation(lg5, ps5[:], Act.Ln)
                # state update per s: alpha_s' = c*a3_s + v_s, then a2 for
                # s before moving to the other s so Exp_0 can start early
                alpha_new = state_pool.tile([128, 2 * K], sdt, tag="alpha")
                last = t + 1 >= T
                if not last:
                    a2n = small_pool.tile([128, 2], f32, tag="a2")
                    u2n = u_pool.tile([128, 2 * K], sdt, tag="u2")
                for s in range(2):
                    nc.vector.scalar_tensor_tensor(
                        out=alpha_new[:, s * K:(s + 1) * K], in0=c_ap(t),
                        scalar=lgbuf[:, 5 * t + s:5 * t + s + 1],
                        in1=v[:, s * K:(s + 1) * K],
                        op0=Alu.mult, op1=Alu.add,
                    )
                    if not last:
                        nc.vector.scalar_tensor_tensor(
                            out=u2n[:, s * K:(s + 1) * K], in0=c_ap(t + 1),
                            scalar=0.0,
                            in1=alpha_new[:, s * K:(s + 1) * K],
                            op0=Alu.bypass, op1=Alu.mult,
                            accum_out=a2n[:, s:s + 1],
                        )
                alpha = alpha_new
                if not last:
                    a2, u2 = a2n, u2n

        # normalize all outputs at once: log_py[t, o] = lpo[t, o] - lq[t]
        lg3 = lgbuf[:].rearrange("p (t f) -> p t f", f=5)
        nc.vector.tensor_tensor(
            out=outbuf[:].rearrange("p (t o) -> p t o", o=2),
            in0=lg3[:, :, 2:4],
            in1=lg3[:, :, 4:5].broadcast_to([128, T, 2]),
            op=Alu.subtract,
        )
        nc.sync.dma_start(out.ap(), outbuf[:])

    nc.compile()
    return nc


def _prep_inputs(corr, kc, problem, A, trans_logits, obs_logits_problem, init_logits, T):
    corr = np.asarray(corr).astype(np.int64)
    kc = np.asarray(kc).astype(np.int64)
    problem = np.asarray(problem).astype(np.int64)
    taba, G, L, alpha0 = _host_tables(
        np.asarray(A), np.asarray(trans_logits),
        np.asarray(obs_logits_problem), np.asarray(init_logits))

    in_maps = []
    for i in range(N_CORES):
        sl = slice(i * BL, (i + 1) * BL)
        kc_l = kc[sl, :T]  # [128, T]
        pp_l = problem[sl, :T]
        cr_l = corr[sl, :T]
        # dense per-(student,step) stream, grouped by state s:
        # strm[.., 5s:5s+5] = [M4[s], M4[2+s], L4[s], L4[2+s], 0]
        Gk = G[kc_l]  # [128,T,4]
        Lp = L[pp_l]  # [128,T,2,2]
        OLL = np.take_along_axis(
            Lp, cr_l[:, :, None, None], axis=3)[:, :, :, 0]  # [128,T,2]
        M4 = (Gk.reshape(BL, T, 2, 2) + OLL[:, :, None, :]).reshape(BL, T, 4)
        L4 = Lp.transpose(0, 1, 3, 2).reshape(BL, T, 4)  # L4[2o+s] = L[s,o]
        Z = np.zeros((BL, T), np.float32)
        strm = np.stack(
            [M4[..., 0], M4[..., 2], L4[..., 0], L4[..., 2], Z,
             M4[..., 1], M4[..., 3], L4[..., 1], L4[..., 3], Z],
            axis=2).astype(np.float32).reshape(BL, T * 10)
        # gather flat order: i = j*128 + p  ->  idx = kc_l[p, j]
        kcw = _wrap_idx(kc_l.T.ravel())
        Th = min(2 * _CHUNK, T)
        im = {
            "taba": taba, "strm": strm, "kcw": kcw, "alpha0": alpha0,
            "strmh": np.ascontiguousarray(strm[:, 0:Th * 10]),
        }
        # host-pregathered A slabs for the first _PREFETCH chunks: matches
        # the dma_gather output layout slab[p, j, :] = taba[kc_l[p, j]]
        n_dense = min(_PREFETCH, (T + _CHUNK - 1) // _CHUNK)
        Td = min(2, T)
        for n in range(n_dense):
            t0 = n * _CHUNK
            tcn = min(_CHUNK, T - t0)
            d = taba[kc_l[:, t0:t0 + tcn]]  # [128, tcn, 128]
            if n == 0 and tcn > Td:
                im["slabd0h"] = np.ascontiguousarray(
                    d[:, 0:Td].reshape(BL, Td * 128))
                d = d[:, Td:]
                tcn -= Td
            elif n == 0:
                im["slabd0h"] = np.ascontiguousarray(
                    d.reshape(BL, tcn * 128))
            im[f"slabd{n}"] = np.ascontiguousarray(
                d.reshape(BL, tcn * 128))
        in_maps.append(im)
    return in_maps


def kernel(corr, kc, problem, A, trans_logits, obs_logits_problem, init_logits,
           _T=None, _trace=False):
    T = _T or T_FULL
    nc = _emit_program(T, min(_CHUNK, T))
    in_maps = _prep_inputs(corr, kc, problem, A, trans_logits,
                           obs_logits_problem, init_logits, T)

    from concourse.bass_utils import run_bass_kernel_spmd
    res = run_bass_kernel_spmd(nc, in_maps, core_ids=list(range(N_CORES)),
                               trace=_trace)
    outs = [r["out"].reshape(BL, T, 2) for r in res.results]
    full = np.concatenate(outs, axis=0).astype(np.float32)
    kernel.last_results = res
    return full


if __name__ == "__main__":
    pass



# revision 16
# speedup vs baseline: 1.1066x; 1.0002x over previous
"""Trainium2 Bass kernel for the BKT (multi-HMM knowledge tracing) forward model.

Reformulated recursion (validated in proto.py):
  state  γ(t) = α(t) − a3(t−1)            [128 students × (s,k) = 200]
  chain  x4(t) = u4(t) + κ_{t−1}·a3(t−1)  [stt w4, DVE]
         ps2(t)[s'] = Σ_s exp(x4[s,s'])   [2× ACT Exp w2 with accum_out]
         a3(t) = ln(ps2)                  [ACT Ln w2]
  off    pd(t) = a3(t−1) − a3(t)          [Pool tt w2]
         γ(t+1) = cinv_t ⊙ (γ(t)+pd(t))   [2× DVE stt w100]
         r'(t+2) = <Q_{t+1}, γ(t+1)> + σ_{t+1}·a3(t)
             [2× DVE affine_mul_reduce: accum = Σ(γ+a3)·Q = <Q,γ> + a3·ΣQ
              and ΣQ_{t+1} = σ_{t+1}, so the seed rides the bias slot]
         u4(t+2) = w4(t+2) + r'(t+2)      [Pool tt w4]
  where Q_t = c_{t+1}⊙(1−c_t), σ_t = ΣQ_t, κ_t = 1−σ_t, all host streams.
  (tensor_tensor_reduce and ACT accum_out crash this runtime — avoided.)
Outputs deferred to one batched tail:
  D(j) = (r'_1−r'_0)(j) + κ_{j−1}·(a3_1−a3_0)(j−1),
  out[t,o] = L0[t,o] + ln(1+e^{dL[t,o]+D}) − ln(1+e^{D}).
Step 0/1 warm-started on host (uploads γ(1), a3(0), u4(1..2), r'(2), D(0..1)).
No gathers: Q/cinv streamed dense from HBM, chunked + double buffered.
"""

import os
from contextlib import ExitStack

import numpy as np

N_PROBLEMS = 10000
N_KCS = 100
BATCH = 1024
T_FULL = 500
N_CORES = 8
BL = BATCH // N_CORES  # 128 students per core

_CH = 32       # time steps per Q/cinv stream chunk
_PREF = 2      # chunks prefetched ahead
_SMALL_ON_POOL = True


def _log_softmax(x, axis):
    x = x.astype(np.float64)
    m = x.max(axis=axis, keepdims=True)
    e = np.exp(x - m)
    return (x - m) - np.log(e.sum(axis=axis, keepdims=True))


def _setup_act_tables():
    """Force the 'natural_log_exp_and_others' ACT table set to be chosen for
    both Exp and Ln so no ACT_TABLE_LOAD appears mid-loop."""
    import glob
    import json
    import tempfile

    if os.environ.get("_BKT_ACT_TABLES"):
        return
    from neuronxcc.driver.Job import Job  # pyright: ignore[reportMissingImports]
    from neuronxcc.driver.jobs.support.FindActInfo import (  # pyright: ignore[reportMissingImports]
        findActInfoFile,
    )

    src = findActInfoFile(Job.getPackageDir(), "gen3")
    d = json.load(open(src))
    d["act_func_sets"] = sorted(
        d["act_func_sets"],
        key=lambda s: s["name"] != "natural_log_exp_and_others")
    tmp = tempfile.mkdtemp(prefix="bkt_act_")
    with open(tmp + "/act_info.json", "w") as f:
        json.dump(d, f)
    for p in glob.glob(os.path.dirname(src) + "/*"):
        b = os.path.basename(p)
        if b != "act_info.json":
            os.symlink(p, tmp + "/" + b)
    os.environ["BASS_ACT_ROOT_JSON_PATH"] = tmp + "/act_info.json"
    os.environ["_BKT_ACT_TABLES"] = "1"

    import concourse.bacc as bacc_mod
    import concourse.mybir as mybir

    def tables(arch):
        return {
            e["name"]: {mybir.ActivationFunctionType.from_pwp(v)
                        for v in e["act"].keys()}
            for e in d["act_func_sets"]
        }

    bacc_mod.get_activation_tables = tables


def _emit_program(T):
    import concourse.mybir as mybir
    import concourse.tile as tile
    from concourse import bacc, library_config

    _setup_act_tables()

    f32 = mybir.dt.float32
    Alu = mybir.AluOpType
    Act = mybir.ActivationFunctionType
    K = N_KCS

    nc = bacc.Bacc("TRN2", target_bir_lowering=False, debug=False)

    # DRAM inputs
    f16 = mybir.dt.float16
    qs = nc.dram_tensor("qs", [BL, T * K], f16, kind="ExternalInput")
    cinvs = nc.dram_tensor("cinvs", [BL, T * K], f16, kind="ExternalInput")
    sigs = nc.dram_tensor("sigs", [BL, T], f32, kind="ExternalInput")
    sigd = nc.dram_tensor("sigd", [BL, T], f32, kind="ExternalInput")
    w4s = nc.dram_tensor("w4s", [BL, 4 * T], f32, kind="ExternalInput")
    kaps = nc.dram_tensor("kaps", [BL, T], f32, kind="ExternalInput")
    l0s_d = nc.dram_tensor("l0s", [BL, 2 * T], f32, kind="ExternalInput")
    dls_d = nc.dram_tensor("dls", [BL, 2 * T], f32, kind="ExternalInput")
    gammaw = nc.dram_tensor("gammaw", [BL, 2 * K], f16, kind="ExternalInput")
    a30w = nc.dram_tensor("a30w", [BL, 2], f32, kind="ExternalInput")
    u4w = nc.dram_tensor("u4w", [BL, 8], f32, kind="ExternalInput")
    rp2w = nc.dram_tensor("rp2w", [BL, 2], f32, kind="ExternalInput")
    d01w = nc.dram_tensor("d01w", [BL, 2], f32, kind="ExternalInput")
    out_d = nc.dram_tensor("out", [BL, 2 * T], f32, kind="ExternalOutput")

    n_chunks = (T + _CH - 1) // _CH

    with ExitStack() as ctx:
        tc = ctx.enter_context(tile.TileContext(nc))
        if _SMALL_ON_POOL:
            nc.gpsimd.load_library(library_config.standard)

        fixed = ctx.enter_context(tc.tile_pool(name="fixed", bufs=1))
        slabp = ctx.enter_context(tc.tile_pool(name="slabs", bufs=_PREF + 1))
        gpool = ctx.enter_context(tc.tile_pool(name="gamma", bufs=2))
        x4p = ctx.enter_context(tc.tile_pool(name="x4", bufs=3))
        sm = ctx.enter_context(tc.tile_pool(name="sm", bufs=3))

        # --- fixed tiles + warm uploads (issue order = DMA drain order) ---
        gamma = gpool.tile([BL, 2 * K], f16, tag="gamma")
        nc.sync.dma_start(gamma[:], gammaw.ap())
        a3buf = fixed.tile([BL, 2 * T], f32)
        nc.sync.dma_start(a3buf[:, 0:2], a30w.ap())
        u4buf = fixed.tile([BL, 4 * T], f32)
        nc.sync.dma_start(u4buf[:, 4:12], u4w.ap())
        r2buf = fixed.tile([BL, 2 * T], f32)
        nc.sync.dma_start(r2buf[:, 4:6], rp2w.ap())
        kap = fixed.tile([BL, T], f32)
        nc.sync.dma_start(kap[:], kaps.ap())
        sig = fixed.tile([BL, T], f32)
        nc.sync.dma_start(sig[:], sigs.ap())
        sigdt = fixed.tile([BL, T], f32)
        nc.sync.dma_start(sigdt[:], sigd.ap())

        qslab = [None] * n_chunks
        cislab = [None] * n_chunks

        def issue_chunk(n):
            t0 = n * _CH
            w = min(_CH, T - t0) * K
            qt = slabp.tile([BL, _CH, K], f16, tag="qsl")
            nc.sync.dma_start(qt[:].rearrange("p a b -> p (a b)")[:, 0:w],
                              qs.ap()[:, t0 * K:t0 * K + w])
            ct = slabp.tile([BL, _CH, K], f16, tag="cisl")
            nc.sync.dma_start(ct[:].rearrange("p a b -> p (a b)")[:, 0:w],
                              cinvs.ap()[:, t0 * K:t0 * K + w])
            qslab[n], cislab[n] = qt, ct

        for n in range(min(_PREF, n_chunks)):
            issue_chunk(n)

        w4b = fixed.tile([BL, 4 * T], f32)
        nc.sync.dma_start(w4b[:], w4s.ap())
        Dbuf = fixed.tile([BL, T], f32)
        nc.sync.dma_start(Dbuf[:, 0:2], d01w.ap())
        l0b = fixed.tile([BL, 2 * T], f32)
        nc.sync.dma_start(l0b[:], l0s_d.ap())
        dlb = fixed.tile([BL, 2 * T], f32)
        nc.sync.dma_start(dlb[:], dls_d.ap())

        junk = fixed.tile([BL, 2 * K], f16)

        smalls = nc.gpsimd if _SMALL_ON_POOL else nc.vector

        # --- main loop: chain t = 1..T-2; off-chain pipelined one iter late ---
        gamma_box = [gamma]

        def emit_off(t):
            n = t // _CH
            pd2t = sm.tile([BL, 2], f32, tag="pd2")
            smalls.tensor_tensor(
                out=pd2t[:], in0=a3buf[:, 2 * (t - 1):2 * t],
                in1=a3buf[:, 2 * t:2 * t + 2], op=Alu.subtract)
            gold = gamma_box[0]
            gnew = gpool.tile([BL, 2 * K], f16, tag="gamma")
            ci = cislab[n][:, t % _CH, :]
            qv = qslab[n][:, t % _CH, :]
            for s in range(2):
                nc.vector.scalar_tensor_tensor(
                    out=gnew[:, s * K:(s + 1) * K],
                    in0=gold[:, s * K:(s + 1) * K],
                    scalar=pd2t[:, s:s + 1],
                    in1=ci, op0=Alu.add, op1=Alu.mult,
                )
            h2t = sm.tile([BL, 2], f32, tag="h2")
            nc.scalar.activation(h2t[:], a3buf[:, 2 * t:2 * t + 2], Act.Copy,
                                 scale=sig[:, t:t + 1])
            for s in range(2):
                # raw dot: accum_out = <Q_{t+1}, γ_s(t+1)>
                nc.vector.scalar_tensor_tensor(
                    out=junk[:, s * K:(s + 1) * K],
                    in0=gnew[:, s * K:(s + 1) * K], scalar=0.0, in1=qv,
                    op0=Alu.bypass, op1=Alu.mult,
                    accum_out=r2buf[:, 2 * (t + 2) + s:2 * (t + 2) + s + 1],
                )
            u4a = sm.tile([BL, 4], f32, tag="u4a")
            smalls.tensor_tensor(
                out=u4a[:].rearrange("p (s sp) -> p s sp", s=2),
                in0=w4b[:, 4 * (t + 2):4 * (t + 2) + 4]
                    .rearrange("p (s sp) -> p s sp", s=2),
                in1=r2buf[:, 2 * (t + 2):2 * (t + 2) + 2]
                    .rearrange("p (s o) -> p s o", s=2).broadcast_to([BL, 2, 2]),
                op=Alu.add,
            )
            smalls.tensor_tensor(
                out=u4buf[:, 4 * (t + 2):4 * (t + 2) + 4]
                    .rearrange("p (s sp) -> p s sp", s=2),
                in0=u4a[:].rearrange("p (s sp) -> p s sp", s=2),
                in1=h2t[:].rearrange("p (s o) -> p s o", s=2)
                    .broadcast_to([BL, 2, 2]),
                op=Alu.add,
            )
            gamma_box[0] = gnew

        pending = None
        for t in range(1, T - 1):
            n = t // _CH
            m = n + _PREF - 1
            if t % _CH == 0 and m < n_chunks and qslab[m] is None:
                issue_chunk(m)

            # CHAIN: x4(t) = κ_{t-1}·a3(t-1) + u4(t)
            x4t = x4p.tile([BL, 4], f32, tag="x4")
            nc.vector.scalar_tensor_tensor(
                out=x4t[:].rearrange("p (s sp) -> p s sp", s=2),
                in0=a3buf[:, 2 * (t - 1):2 * t]
                    .rearrange("p (s o) -> p s o", s=2).broadcast_to([BL, 2, 2]),
                scalar=kap[:, t:t + 1],
                in1=u4buf[:, 4 * t:4 * t + 4].rearrange("p (s sp) -> p s sp", s=2),
                op0=Alu.mult, op1=Alu.add,
            )
            if pending is not None:
                emit_off(pending)
                pending = None
            e4t = sm.tile([BL, 4], f32, tag="e4")
            nc.scalar.activation(e4t[:], x4t[:], Act.Exp)
            ps2t = sm.tile([BL, 2], f32, tag="ps2")
            nc.vector.tensor_tensor(
                out=ps2t[:], in0=e4t[:, 0:2], in1=e4t[:, 2:4], op=Alu.add)
            nc.scalar.activation(a3buf[:, 2 * t:2 * t + 2], ps2t[:], Act.Ln)
            if t <= T - 3:
                pending = t
        if pending is not None:
            emit_off(pending)

        # --- deferred output tail ---
        da = fixed.tile([BL, T], f32)
        a3v = a3buf[:].rearrange("p (t s) -> p t s", s=2)
        nc.vector.tensor_tensor(
            out=da[:, 0:T - 1].rearrange("p (t o) -> p t o", o=1),
            in0=a3v[:, 0:T - 1, 1:2], in1=a3v[:, 0:T - 1, 0:1],
            op=Alu.subtract)
        dr = fixed.tile([BL, T], f32)
        r2v = r2buf[:].rearrange("p (t s) -> p t s", s=2)
        nc.vector.tensor_tensor(
            out=dr[:, 2:T].rearrange("p (t o) -> p t o", o=1),
            in0=r2v[:, 2:T, 1:2], in1=r2v[:, 2:T, 0:1], op=Alu.subtract)
        m1 = fixed.tile([BL, T], f32)
        nc.vector.tensor_tensor(
            out=m1[:, 2:T], in0=kap[:, 2:T], in1=da[:, 1:T - 1], op=Alu.mult)
        m2 = fixed.tile([BL, T], f32)
        nc.vector.tensor_tensor(
            out=m2[:, 2:T], in0=sigdt[:, 2:T], in1=da[:, 0:T - 2], op=Alu.mult)
        d1t = fixed.tile([BL, T], f32)
        nc.vector.tensor_tensor(
            out=d1t[:, 2:T], in0=dr[:, 2:T], in1=m1[:, 2:T], op=Alu.add)
        nc.vector.tensor_tensor(
            out=Dbuf[:, 2:T], in0=d1t[:, 2:T], in1=m2[:, 2:T], op=Alu.add)
        yD = fixed.tile([BL, 2 * T], f32)
        nc.vector.tensor_tensor(
            out=yD[:].rearrange("p (t o) -> p t o", o=2),
            in0=dlb[:].rearrange("p (t o) -> p t o", o=2),
            in1=Dbuf[:].rearrange("p (t o) -> p t o", o=1)
                .broadcast_to([BL, T, 2]),
            op=Alu.add)
        e2 = fixed.tile([BL, 2 * T], f32)
        nc.scalar.activation(e2[:], yD[:], Act.Exp)
        l2 = fixed.tile([BL, 2 * T], f32)
        nc.scalar.activation(l2[:], e2[:], Act.Ln, bias=1.0)
        ed = fixed.tile([BL, T], f32)
        nc.scalar.activation(ed[:], Dbuf[:], Act.Exp)
        ld = fixed.tile([BL, T], f32)
        nc.scalar.activation(ld[:], ed[:], Act.Ln, bias=1.0)
        outb = fixed.tile([BL, 2 * T], f32)
        nc.vector.tensor_tensor(
            out=outb[:].rearrange("p (t o) -> p t o", o=2),
            in0=l2[:].rearrange("p (t o) -> p t o", o=2),
            in1=ld[:].rearrange("p (t o) -> p t o", o=1)
                .broadcast_to([BL, T, 2]),
            op=Alu.subtract)
        nc.vector.tensor_tensor(out=outb[:], in0=outb[:], in1=l0b[:], op=Alu.add)
        nc.sync.dma_start(out_d.ap(), outb[:])

    nc.compile()
    return nc


def _prep_inputs(corr, kc, problem, A, trans_logits, obs_logits_problem,
                 init_logits, T):
    corr = np.asarray(corr).astype(np.int64)[:, :T]
    kc = np.asarray(kc).astype(np.int64)[:, :T]
    problem = np.asarray(problem).astype(np.int64)[:, :T]
    A = np.asarray(A).astype(np.float64)
    K = N_KCS

    log_t = _log_softmax(np.asarray(trans_logits), axis=1)
    G = A @ log_t.reshape(K, 4)                       # [P,4], j = 2 s' + s
    L = _log_softmax(np.asarray(obs_logits_problem), axis=2)
    la0 = _log_softmax(np.asarray(init_logits), axis=1)

    in_maps = []
    for i in range(N_CORES):
        sl = slice(i * BL, (i + 1) * BL)
        kc_l, pp_l, cr_l = kc[sl], problem[sl], corr[sl]
        c = A[kc_l]                                   # [BL,T,K]
        cinv = 1.0 - c
        Q = c[:, 1:] * cinv[:, :-1]                   # Q[:,j] = c_{j+1}*cinv_j
        sigma = Q.sum(-1)
        kappa = 1.0 - sigma
        OLL = np.take_along_axis(
            L[pp_l], cr_l[:, :, None, None], axis=3)[:, :, :, 0]  # [BL,T,2]
        Gk = G[kc_l]
        w4 = np.stack([Gk[..., 0] + OLL[..., 0], Gk[..., 2] + OLL[..., 0],
                       Gk[..., 1] + OLL[..., 1], Gk[..., 3] + OLL[..., 1]],
                      axis=2)                         # [BL,T,4] (s,s')
        Lp = L[pp_l]
        l0 = np.ascontiguousarray(Lp[:, :, 0, :]).reshape(BL, 2 * T)
        dl = np.ascontiguousarray(Lp[:, :, 1, :] - Lp[:, :, 0, :]).reshape(BL, 2 * T)

        # warm start
        alpha0 = np.broadcast_to(la0.T.reshape(1, 2, K), (BL, 2, K))
        a2_0 = np.einsum('bk,bsk->bs', c[:, 0], alpha0)
        x40 = w4[:, 0].reshape(BL, 2, 2) + a2_0[:, :, None]
        a30 = np.log(np.exp(x40).sum(axis=1))
        alpha1 = cinv[:, 0][:, None, :] * alpha0 + c[:, 0][:, None, :] * a30[:, :, None]
        gamma1 = alpha1 - a30[:, :, None]
        a2_1 = np.einsum('bk,bsk->bs', c[:, 1], gamma1) + a30
        u4_1 = w4[:, 1].reshape(BL, 2, 2) + a2_1[:, :, None] \
            - kappa[:, 0][:, None, None] * a30[:, :, None]
        rp2_raw = np.einsum('bk,bsk->bs', Q[:, 1], gamma1)
        rp2 = rp2_raw + sigma[:, 1][:, None] * a30
        u4_2 = w4[:, 2].reshape(BL, 2, 2) + rp2[:, :, None]

        # iteration-shifted streams
        kap_it = np.zeros((BL, T)); kap_it[:, 1:] = kappa[:, 0:T - 1]
        sig_it = np.zeros((BL, T)); sig_it[:, 1:T - 2] = sigma[:, 2:T - 1]
        sig_d = np.zeros((BL, T)); sig_d[:, 2:] = sigma[:, 1:T - 1]
        q_it = np.zeros((BL, T, K)); q_it[:, 1:T - 2] = Q[:, 2:T - 1]
        cinv_it = np.zeros((BL, T, K)); cinv_it[:, 1:T - 2] = cinv[:, 1:T - 2]

        f = np.float32
        im = {
            "qs": q_it.reshape(BL, T * K).astype(np.float16),
            "cinvs": cinv_it.reshape(BL, T * K).astype(np.float16),
            "w4s": w4.reshape(BL, 4 * T).astype(f),
            "kaps": kap_it.astype(f),
            "sigs": sig_it.astype(f),
            "sigd": sig_d.astype(f),
            "l0s": l0.astype(f),
            "dls": dl.astype(f),
            "gammaw": gamma1.reshape(BL, 2 * K).astype(np.float16),
            "a30w": a30.astype(f),
            "u4w": np.concatenate([u4_1.reshape(BL, 4), u4_2.reshape(BL, 4)],
                                  axis=1).astype(f),
            "rp2w": rp2_raw.astype(f),
            "d01w": np.stack([a2_0[:, 1] - a2_0[:, 0],
                              a2_1[:, 1] - a2_1[:, 0]], axis=1).astype(f),
        }
        in_maps.append(im)
    return in_maps


def kernel(corr, kc, problem, A, trans_logits, obs_logits_problem, init_logits,
           _T=None, _trace=False):
    T = _T or T_FULL
    nc = _emit_program(T)
    in_maps = _prep_inputs(corr, kc, problem, A, trans_logits,
                           obs_logits_problem, init_logits, T)

    from concourse.bass_utils import run_bass_kernel_spmd
    res = run_bass_kernel_spmd(nc, in_maps, core_ids=list(range(N_CORES)),
                               trace=_trace)
    outs = [r["out"].reshape(BL, T, 2) for r in res.results]
    full = np.concatenate(outs, axis=0).astype(np.float32)
    kernel.last_results = res
    return full


if __name__ == "__main__":
    pass
